# revision 19
# baseline (speedup 1.0000x reference)
"""Trainium2 kernel for nn_Net_68994354643186 (3-layer TransformerConv GNN).

Strategy (8 NeuronCores, node/data-parallel, edge-cut partitioning):
  - Nodes are partitioned into 8 shards of 6250 (padded to 6272 = 49*128).
  - Edges are owned by the core that owns their *destination* node, grouped
    into 49 windows of 128 destination nodes per core, with fixed padded
    capacities (13 tiles of "lo" src + 7 tiles of "hi" src per window; lo/hi
    split because dma_gather indices are int16).
  - ONE SPMD NEFF runs the whole network:
      per layer: replicated GEMM computes the k|v table for ALL nodes
      (avoids halo exchange for source gathers), a local GEMM computes q|s
      for the core's own nodes; the edge phase gathers k|v rows by src and
      q rows by dst (batched SWDGE dma_gather), computes exp(q.k/sqrt(D))
      per edge (no max subtraction -- scores are O(6) for this data), and
      aggregates numerator and denominator per destination node with
      selection-matrix matmuls accumulated in PSUM; normalization, root
      skip and leaky-relu happen at the node level.  Between layers the
      (transposed) node features are AllGathered so the next layer's
      replicated GEMM sees all nodes.  log_softmax of layer 3 runs on
      device; host only concatenates the 8 output shards.

Self-contained: hardcodes all shapes; no sibling imports.
"""

import sys

sys.path.insert(0, "/opt/trn_rl_repo")

import numpy as np
import ml_dtypes

BF16 = ml_dtypes.bfloat16

N_NODES = 50000
N_EDGES = 800000
N_CORES = 8
LOC = N_NODES // N_CORES          # 6250 local nodes per core
NW = 49                           # windows of 128 dst nodes per core
LPAD = NW * 128                   # 6272 padded local nodes
SPLIT = 32768                     # int16 index split for the kv table
CA_T, CB_T = 13, 7                # window capacity: lo/hi tiles of 128 edges
CAP_A, CAP_B = CA_T * 128, CB_T * 128
NT = CA_T + CB_T                  # 20 edge tiles per window
CAP = NT * 128                    # 2560 edge slots per window
LEAKY_ALPHA = 0.1

# (cin, H, D, hd, kv_pad_row_f32, q_pad_row_f32)
# kv_pad/q_pad are row lengths in bf16 elements; row bytes must be a
# multiple of 256 for dma_gather.
LAYERS = [
    (130, 4, 50, 200, 512, 256),
    (200, 4, 25, 100, 256, 128),
    (100, 4, 10, 40, 128, 128),
]

_COMPILED = {}


# --------------------------------------------------------------------------
# device program
# --------------------------------------------------------------------------

def _build_program(caw, cbw):
    import concourse.bass as bass
    import concourse.bacc as bacc
    import concourse.mybir as mybir
    import concourse.tile as tile
    from concourse.masks import make_identity

    f32 = mybir.dt.float32
    bf16 = mybir.dt.bfloat16
    i16 = mybir.dt.int16
    i32 = mybir.dt.int32

    nc = bacc.Bacc("TRN2", num_devices=N_CORES, num_swdge_queues=4)

    xT = nc.dram_tensor("xt", [131, LPAD], bf16, kind="ExternalInput")
    w_in = {}
    for li, (cin, H, D, hd, kvp, qp) in enumerate(LAYERS):
        w_in[("kv", li)] = nc.dram_tensor(
            f"wkv{li}", [cin + 1, 2 * hd], bf16, kind="ExternalInput")
        w_in[("qs", li)] = nc.dram_tensor(
            f"wqs{li}", [cin + 1, 2 * hd], bf16, kind="ExternalInput")
    # per-window column offsets into the packed edge-index tensor:
    # [kvlo (caw*8) | kvhi (cbw*8) | q (ntw*8) | widx (ntw, bf16 bits)]
    ntw = [a + b for a, b in zip(caw, cbw)]
    wcols = [a * 8 + b * 8 + n * 8 + n for a, b, n in zip(caw, cbw, ntw)]
    woff = [0]
    for v in wcols:
        woff.append(woff[-1] + v)
    NIDX = woff[-1]
    NTMAX = max(ntw)
    eidx_in = nc.dram_tensor("eidx", [128, NIDX], i16, kind="ExternalInput")
    out_t = nc.dram_tensor("out", [LPAD, 40], f32, kind="ExternalOutput")

    RG = [list(range(N_CORES))]

    with tile.TileContext(nc) as tc:
        with (
            tc.tile_pool(name="const", bufs=1) as cpool,
            tc.tile_pool(name="wpool", bufs=1) as wpool,
            tc.tile_pool(name="lhs", bufs=4) as lhs,
            tc.tile_pool(name="gout", bufs=4) as gout,
            tc.tile_pool(name="kvw", bufs=3) as kvw,
            tc.tile_pool(name="qw", bufs=3) as qw,
            tc.tile_pool(name="sw", bufs=3) as sw,
            tc.tile_pool(name="cw", bufs=2) as cw,
            tc.tile_pool(name="how", bufs=3) as how,
            tc.tile_pool(name="psg", bufs=2, space="PSUM") as psg,
            tc.tile_pool(name="psa", bufs=4, space="PSUM") as psa,
            tc.tile_pool(name="pst", bufs=2, space="PSUM") as pst,
            tc.tile_pool(name="dram", bufs=1, space="DRAM") as dram,
        ):
            # constants
            iota_i = cpool.tile([128, 128], i32, tag="iota_i")
            nc.gpsimd.iota(iota_i[:], pattern=[[1, 128]], base=0, channel_multiplier=0)
            iota_f = cpool.tile([128, 128], bf16, tag="iota_f")
            nc.vector.tensor_copy(out=iota_f[:], in_=iota_i[:])
            ident = cpool.tile([128, 128], f32, tag="ident")
            make_identity(nc, ident[:])

            # persistent DRAM buffers
            kv_t = [dram.tile([N_CORES * LPAD, LAYERS[l][4]], bf16, tag=f"kv{l}",
                              name=f"kv{l}") for l in range(3)]
            q_t = [dram.tile([LPAD, LAYERS[l][5]], bf16, tag=f"q{l}", name=f"q{l}")
                   for l in range(3)]
            s_t = [dram.tile([LPAD, LAYERS[l][3]], f32, tag=f"s{l}", name=f"s{l}")
                   for l in range(3)]
            # transposed local features (+1 ones row) per layer boundary
            hT = [None,
                  dram.tile([LAYERS[1][0] + 1, LPAD], bf16, tag="hT1", name="hT1"),
                  dram.tile([LAYERS[2][0] + 1, LPAD], bf16, tag="hT2", name="hT2")]
            xg = [dram.tile([N_CORES, LAYERS[0][0] + 1, LPAD], bf16, tag="xg0",
                            name="xg0", addr_space="Shared"),
                  dram.tile([N_CORES, LAYERS[1][0] + 1, LPAD], bf16, tag="xg1",
                            name="xg1", addr_space="Shared"),
                  dram.tile([N_CORES, LAYERS[2][0] + 1, LPAD], bf16, tag="xg2",
                            name="xg2", addr_space="Shared")]
            xloc = dram.tile([131, LPAD], bf16, tag="xloc")

            # stage layer-1 features and AllGather them
            nc.sync.dma_start(out=xloc[:], in_=xT.ap())
            nc.gpsimd.collective_compute(
                "AllGather", mybir.AluOpType.bypass, replica_groups=RG,
                ins=[xloc[:].opt()], outs=[xg[0][:].opt()])

            for li, (cin, H, D, hd, kvp, qp) in enumerate(LAYERS):
                KF0 = min(cin + 1, 128)
                K1 = cin + 1 - KF0
                loc_buf = xT.ap() if li == 0 else hT[li][:]
                gath = xg[li]

                # ---- weights to SBUF ----
                wkv0 = wpool.tile([KF0, 2 * hd], bf16, tag="wkv0")
                nc.sync.dma_start(out=wkv0[:], in_=w_in[("kv", li)].ap()[0:KF0, :])
                wqs0 = wpool.tile([KF0, 2 * hd], bf16, tag="wqs0")
                nc.sync.dma_start(out=wqs0[:], in_=w_in[("qs", li)].ap()[0:KF0, :])
                if K1:
                    wkv1 = wpool.tile([K1, 2 * hd], bf16, tag="wkv1")
                    nc.sync.dma_start(out=wkv1[:], in_=w_in[("kv", li)].ap()[KF0:cin + 1, :])
                    wqs1 = wpool.tile([K1, 2 * hd], bf16, tag="wqs1")
                    nc.sync.dma_start(out=wqs1[:], in_=w_in[("qs", li)].ap()[KF0:cin + 1, :])

                # ---- k|v GEMM over ALL nodes (replicated); node tiles are
                # processed in pairs so the bf16 lhsT loads use 512B rows ----
                for c in range(N_CORES):
                    for tp in range(0, NW, 2):
                        npair = min(2, NW - tp)
                        cl = slice(tp * 128, (tp + npair) * 128)
                        a0 = lhs.tile([128, 256], bf16, tag="a0")
                        nc.sync.dma_start(out=a0[:KF0, :npair * 128],
                                          in_=gath[c, 0:KF0, cl])
                        if K1:
                            a1 = lhs.tile([K1, 256], bf16, tag="a1")
                            nc.sync.dma_start(out=a1[:, :npair * 128],
                                              in_=gath[c, KF0:cin + 1, cl])
                        kvs = 2 * hd if 2 * hd * 2 >= 512 else kvp
                        o = gout.tile([128, 2, kvs], bf16, tag="go")
                        for j in range(npair):
                            jc = slice(j * 128, (j + 1) * 128)
                            ps = psg.tile([128, 2 * hd], f32, tag="psg")
                            nc.tensor.matmul(ps[:], lhsT=a0[:KF0, jc], rhs=wkv0[:],
                                             start=True, stop=(K1 == 0))
                            if K1:
                                nc.tensor.matmul(ps[:], lhsT=a1[:, jc], rhs=wkv1[:],
                                                 start=False, stop=True)
                            nc.vector.tensor_copy(out=o[:, j, 0:2 * hd], in_=ps[:])
                        r0 = (c * NW + tp) * 128
                        nc.sync.dma_start(
                            out=kv_t[li][r0:r0 + npair * 128, 0:kvs].rearrange(
                                "(j p) n -> p j n", p=128),
                            in_=o[:, 0:npair, :])

                # ---- q|s GEMM over local nodes ----
                for tp in range(0, NW, 2):
                    npair = min(2, NW - tp)
                    cl = slice(tp * 128, (tp + npair) * 128)
                    b0 = lhs.tile([128, 256], bf16, tag="a0")
                    nc.sync.dma_start(out=b0[:KF0, :npair * 128],
                                      in_=loc_buf[0:KF0, cl])
                    if K1:
                        b1 = lhs.tile([K1, 256], bf16, tag="a1")
                        nc.sync.dma_start(out=b1[:, :npair * 128],
                                          in_=loc_buf[KF0:cin + 1, cl])
                    oq = gout.tile([128, 2, qp], bf16, tag="gq")
                    os_ = gout.tile([128, 2, hd], f32, tag="gs")
                    for j in range(npair):
                        jc = slice(j * 128, (j + 1) * 128)
                        ps = psg.tile([128, 2 * hd], f32, tag="psg")
                        nc.tensor.matmul(ps[:], lhsT=b0[:KF0, jc], rhs=wqs0[:],
                                         start=True, stop=(K1 == 0))
                        if K1:
                            nc.tensor.matmul(ps[:], lhsT=b1[:, jc], rhs=wqs1[:],
                                             start=False, stop=True)
                        nc.vector.tensor_copy(out=oq[:, j, 0:hd], in_=ps[:, 0:hd])
                        nc.vector.tensor_copy(out=os_[:, j, :], in_=ps[:, hd:2 * hd])
                    r0 = tp * 128
                    nc.sync.dma_start(
                        out=q_t[li][r0:r0 + npair * 128, :].rearrange(
                            "(j p) n -> p j n", p=128),
                        in_=oq[:, 0:npair, :])
                    nc.sync.dma_start(
                        out=s_t[li][r0:r0 + npair * 128, :].rearrange(
                            "(j p) n -> p j n", p=128),
                        in_=os_[:, 0:npair, :])

                # ---- edge phase: one window of 128 dst nodes at a time ----
                qn = [0]

                def nextq():
                    qn[0] += 1
                    return qn[0] % 4

                for w in range(NW):
                    CA_W, CB_W = caw[w], cbw[w]
                    NT_W = CA_W + CB_W
                    eix = sw.tile([128, (NTMAX * 17 + 1)], i16, tag="eix")
                    nc.sync.dma_start(out=eix[:, 0:wcols[w]],
                                      in_=eidx_in.ap()[:, woff[w]:woff[w + 1]])
                    c0, c1, c2 = CA_W * 8, (CA_W + CB_W) * 8, \
                        (CA_W + CB_W + NT_W) * 8
                    ilo = eix[:, 0:c0]
                    ihi = eix[:, c0:c1]
                    iq = eix[:, c1:c2]
                    wx = eix[:, c2:c2 + NT_W].bitcast(bf16)

                    # dma_gather is limited to <=1024 indices per call
                    # (SWDGE ring depth); issue in chunks of <=7 tiles.
                    GMAX = 7
                    kvt = kvw.tile([128, NTMAX, kvp], bf16, tag="kvt")
                    for t0 in range(0, CA_W, GMAX):
                        tn = min(GMAX, CA_W - t0)
                        nc.gpsimd.dma_gather(
                            kvt[:, t0:t0 + tn, :], kv_t[li][:],
                            ilo[:, t0 * 8:(t0 + tn) * 8], tn * 128, tn * 128, kvp,
                            queue_num=nextq())
                    for t0 in range(0, CB_W, GMAX):
                        tn = min(GMAX, CB_W - t0)
                        nc.gpsimd.dma_gather(
                            kvt[:, CA_W + t0:CA_W + t0 + tn, :],
                            kv_t[li][SPLIT:, :],
                            ihi[:, t0 * 8:(t0 + tn) * 8], tn * 128, tn * 128, kvp,
                            queue_num=nextq())
                    qt = qw.tile([128, NTMAX, qp], bf16, tag="qt")
                    for t0 in range(0, NT_W, GMAX):
                        tn = min(GMAX, NT_W - t0)
                        nc.gpsimd.dma_gather(
                            qt[:, t0:t0 + tn, :], q_t[li][:],
                            iq[:, t0 * 8:(t0 + tn) * 8], tn * 128, tn * 128, qp,
                            queue_num=nextq())

                    st = sw.tile([128, hd], f32, tag="st")
                    nc.sync.dma_start(out=st[:],
                                      in_=s_t[li][w * 128:(w + 1) * 128, :])

                    # scores = sum_d q*k per head; q is pre-scaled by 1/sqrt(D)
                    tmp = cw.tile([128, NTMAX, hd], bf16, tag="tmp")
                    nc.vector.tensor_mul(tmp[:, 0:NT_W, :], qt[:, 0:NT_W, 0:hd],
                                         kvt[:, 0:NT_W, 0:hd])
                    sc = cw.tile([128, NTMAX, H], f32, tag="sc")
                    nc.vector.reduce_sum(
                        out=sc[:, 0:NT_W, :, None],
                        in_=tmp[:, 0:NT_W, :].rearrange(
                            "p t (h d) -> p t h d", h=H),
                        axis=mybir.AxisListType.X)
                    # wvex = [ exp(s)*v | exp(s) ]
                    wvex = cw.tile([128, NTMAX, hd + 4], bf16, tag="wvex")
                    ex = wvex[:, 0:NT_W, hd:hd + 4]
                    nc.scalar.activation(ex, sc[:, 0:NT_W, :],
                                         mybir.ActivationFunctionType.Exp)
                    nc.vector.tensor_mul(
                        wvex[:, 0:NT_W, 0:hd].rearrange(
                            "p t (h d) -> p t h d", h=H),
                        kvt[:, 0:NT_W, hd:2 * hd].rearrange(
                            "p t (h d) -> p t h d", h=H),
                        ex[:, :, :, None].to_broadcast([128, NT_W, H, D]))
                    # selection matrix: sel[p, t, j] = (widx[p, t] == j)
                    sel = cw.tile([128, NTMAX, 128], bf16, tag="sel")
                    nc.vector.tensor_tensor(
                        out=sel[:, 0:NT_W, :],
                        in0=wx[:, :, None].to_broadcast([128, NT_W, 128]),
                        in1=iota_f[:, None, :].to_broadcast([128, NT_W, 128]),
                        op=mybir.AluOpType.is_equal)
                    # aggregate numerator and denominator per dst node
                    ps = psa.tile([128, hd + 4], f32, tag="psa")
                    for t in range(NT_W):
                        nc.tensor.matmul(ps[:], lhsT=sel[:, t, :],
                                         rhs=wvex[:, t, :],
                                         start=(t == 0), stop=(t == NT_W - 1))
                    den = how.tile([128, 4], f32, tag="den")
                    nc.vector.tensor_scalar(
                        out=den[:], in0=ps[:, hd:hd + 4], scalar1=1e-30,
                        scalar2=None, op0=mybir.AluOpType.add)
                    rec = how.tile([128, 4], f32, tag="rec")
                    nc.vector.reciprocal(out=rec[:], in_=den[:])
                    ho = how.tile([128, hd], f32, tag="ho")
                    nc.vector.tensor_mul(
                        ho[:].rearrange("p (h d) -> p h d", h=H),
                        ps[:, 0:hd].rearrange("p (h d) -> p h d", h=H),
                        rec[:, :, None].to_broadcast([128, H, D]))
                    nc.vector.tensor_add(ho[:], ho[:], st[:])

                    if li < 2:
                        # leaky relu, then transpose into hT for the next layer
                        lk = how.tile([128, hd], f32, tag="lk")
                        nc.vector.tensor_scalar_mul(lk[:], ho[:], LEAKY_ALPHA)
                        nc.vector.tensor_max(ho[:], ho[:], lk[:])
                        for c0 in range(0, hd, 128):
                            cn = min(128, hd - c0)
                            pt = pst.tile([128, 128], f32, tag="pt")
                            nc.tensor.transpose(pt[:cn, :], ho[:, c0:c0 + cn],
                                                ident[:])
                            tt = how.tile([128, 128], bf16, tag="tt")
                            nc.vector.tensor_copy(out=tt[:cn, :], in_=pt[:cn, :])
                            nc.sync.dma_start(
                                out=hT[li + 1][c0:c0 + cn, w * 128:(w + 1) * 128],
                                in_=tt[:cn, :])
                    else:
                        # log_softmax over the 40 output columns
                        mx = how.tile([128, 1], f32, tag="mx")
                        nc.vector.reduce_max(out=mx[:], in_=ho[:],
                                             axis=mybir.AxisListType.X)
                        z = how.tile([128, 40], f32, tag="z")
                        nc.vector.tensor_scalar(
                            out=z[:], in0=ho[:], scalar1=mx[:, 0:1], scalar2=None,
                            op0=mybir.AluOpType.subtract)
                        e = how.tile([128, 40], f32, tag="e")
                        ssum = how.tile([128, 1], f32, tag="ssum")
                        nc.scalar.activation(e[:], z[:],
                                             mybir.ActivationFunctionType.Exp,
                                             accum_out=ssum[:])
                        lg = how.tile([128, 1], f32, tag="lg")
                        nc.scalar.activation(lg[:], ssum[:],
                                             mybir.ActivationFunctionType.Ln)
                        zo = how.tile([128, 40], f32, tag="zo")
                        nc.vector.tensor_scalar(
                            out=zo[:], in0=z[:], scalar1=lg[:, 0:1], scalar2=None,
                            op0=mybir.AluOpType.subtract)
                        nc.sync.dma_start(
                            out=out_t.ap()[w * 128:(w + 1) * 128, :], in_=zo[:])

                if li < 2:
                    # ones row for the bias trick, then AllGather features
                    ones = sw.tile([128, NW], bf16, tag="ones")
                    nc.vector.memset(ones[:], 1.0)
                    nc.sync.dma_start(
                        out=hT[li + 1][hd, :].rearrange("(p c) -> p c", p=128),
                        in_=ones[:])
                    nc.gpsimd.collective_compute(
                        "AllGather", mybir.AluOpType.bypass, replica_groups=RG,
                        ins=[hT[li + 1][:].opt()], outs=[xg[li + 1][:].opt()])

    nc.compile()
    return nc


# --------------------------------------------------------------------------
# host-side preparation
# --------------------------------------------------------------------------

def _prep_edges(src, dst):
    """Build per-core gather indices / window metadata with exact per-window
    tile counts (max over cores, since the SPMD program is shared).  Returns
    None if the graph needs a host fallback."""
    core = dst // LOC
    ldst = dst - core * LOC
    w = ldst >> 7
    wid = (ldst & 127).astype(np.float32)
    srcp = (src // LOC) * LPAD + (src % LOC)
    hi = srcp >= SPLIT
    g = core * NW + w

    deg = np.bincount(dst, minlength=N_NODES)
    if (deg == 0).any():
        return None
    nlo = np.bincount(g[~hi], minlength=N_CORES * NW).reshape(N_CORES, NW)
    nhi = np.bincount(g[hi], minlength=N_CORES * NW).reshape(N_CORES, NW)
    caw = (nlo.max(axis=0) + 127) // 128      # lo tiles per window
    cbw = (nhi.max(axis=0) + 127) // 128      # hi tiles per window
    ntw = caw + cbw
    if ntw.max() > 24:
        return None

    order = np.lexsort((hi, g))
    gs = g[order]
    his = hi[order]
    srcs = srcp[order]
    wids = wid[order]
    lds = ldst[order]

    key = gs * 2 + his
    run_start = np.concatenate(([0], np.flatnonzero(np.diff(key)) + 1))
    starts_full = np.zeros(N_CORES * NW * 2, np.int64)
    starts_full[key[run_start]] = run_start
    rank = np.arange(len(key)) - starts_full[key]

    # per-window slot base offsets (edge slots, in the concatenated layout)
    capA = caw * 128
    capB = cbw * 128
    wbase = np.zeros(NW, np.int64)
    wbase[1:] = np.cumsum(capA + capB)[:-1]
    TOT = int((capA + capB).sum())          # total edge slots per core

    g_w = gs % NW
    slot = wbase[g_w] + np.where(his, capA[g_w] + rank, rank)
    c_arr = gs // NW

    IDX = np.zeros((N_CORES, TOT), np.int16)      # gather row index
    QID = np.zeros((N_CORES, TOT), np.int16)
    WIDX = np.full((N_CORES, TOT), -1.0, np.float32)
    IDX[c_arr, slot] = np.where(his, srcs - SPLIT, srcs).astype(np.int16)
    QID[c_arr, slot] = lds.astype(np.int16)
    WIDX[c_arr, slot] = wids

    def wrap16(a):
        # edge slots i -> [i % 16, i // 16], replicated over 8 part. groups
        n = a.shape[-1]
        a = a.reshape(N_CORES, n // 16, 16).transpose(0, 2, 1)
        a = np.broadcast_to(a[:, None, :, :], (N_CORES, 8, 16, n // 16))
        return a.reshape(N_CORES, 128, n // 16)

    # pack per-window [kvlo | kvhi | q | widx] column blocks
    cols = []
    for w_ in range(NW):
        s0, e0 = wbase[w_], wbase[w_] + capA[w_]
        e1 = e0 + capB[w_]
        cols.append(wrap16(IDX[:, s0:e0]))
        cols.append(wrap16(IDX[:, e0:e1]))
        cols.append(wrap16(QID[:, s0:e1]))
        wx = WIDX[:, s0:e1].reshape(N_CORES, ntw[w_], 128).transpose(0, 2, 1)
        cols.append(np.ascontiguousarray(wx).astype(BF16).view(np.int16))
    eidx = np.ascontiguousarray(np.concatenate(cols, axis=2))
    return {"eidx": eidx, "caw": tuple(int(v) for v in caw),
            "cbw": tuple(int(v) for v in cbw)}


def _prep_weights(inputs):
    ws = {}
    for li, (cin, H, D, hd, kvp, qp) in enumerate(LAYERS):
        s = 1.0 / np.sqrt(np.float32(D))
        wkv = np.zeros((cin + 1, 2 * hd), np.float32)
        wkv[:cin, :hd] = inputs[f"Wk{li + 1}"]
        wkv[cin, :hd] = inputs[f"bk{li + 1}"]
        wkv[:cin, hd:] = inputs[f"Wv{li + 1}"]
        wkv[cin, hd:] = inputs[f"bv{li + 1}"]
        wqs = np.zeros((cin + 1, 2 * hd), np.float32)
        wqs[:cin, :hd] = inputs[f"Wq{li + 1}"] * s
        wqs[cin, :hd] = inputs[f"bq{li + 1}"] * s
        wqs[:cin, hd:] = inputs[f"Ws{li + 1}"]
        wqs[cin, hd:] = inputs[f"bs{li + 1}"]
        ws[f"wkv{li}"] = wkv.astype(BF16)
        ws[f"wqs{li}"] = wqs.astype(BF16)
    return ws


def _host_fallback(x, src, dst, inputs):
    """Pure-numpy reference path (used only if the graph exceeds the
    compiled capacities)."""
    order = np.argsort(dst, kind="stable")
    so, do = src[order], dst[order]
    seg_starts = np.flatnonzero(np.concatenate(([True], do[1:] != do[:-1])))
    seg_ids = do[seg_starts]
    h = x
    for li, (cin, H, D, hd, kvp, qp) in enumerate(LAYERS):
        q = (h @ inputs[f"Wq{li + 1}"] + inputs[f"bq{li + 1}"]).reshape(-1, H, D)
        k = (h @ inputs[f"Wk{li + 1}"] + inputs[f"bk{li + 1}"]).reshape(-1, H, D)
        v = (h @ inputs[f"Wv{li + 1}"] + inputs[f"bv{li + 1}"]).reshape(-1, H, D)
        s = h @ inputs[f"Ws{li + 1}"] + inputs[f"bs{li + 1}"]
        sc = np.einsum("ehd,ehd->eh", q[do], k[so], optimize=True) / np.sqrt(
            np.float32(D))
        m = np.zeros((N_NODES, H), np.float32)
        m[seg_ids] = np.maximum.reduceat(sc, seg_starts, axis=0)
        e = np.exp(sc - m[do])
        den = np.zeros((N_NODES, H), np.float32)
        den[seg_ids] = np.add.reduceat(e, seg_starts, axis=0)
        alpha = e / (den[do] + 1e-16)
        outa = np.zeros((N_NODES, H, D), np.float32)
        outa[seg_ids] = np.add.reduceat(alpha[:, :, None] * v[so], seg_starts,
                                        axis=0)
        h = outa.reshape(N_NODES, hd) + s
        if li < 2:
            h = np.where(h >= 0, h, np.float32(LEAKY_ALPHA) * h)
    m = h.max(axis=1, keepdims=True)
    z = h - m
    return (z - np.log(np.exp(z).sum(axis=1, keepdims=True))).astype(np.float32)


def _run_device(in_maps, caw, cbw, trace=False, trace_cores=None):
    from concourse.bass_utils import run_bass_kernel_spmd

    key = (tuple(caw), tuple(cbw))
    if _COMPILED.get("key") != key:
        _COMPILED["nc"] = _build_program(caw, cbw)
        _COMPILED["key"] = key
    kw = {}
    if trace:
        kw = dict(trace=True)
        if trace_cores is not None:
            kw["trace_cores"] = trace_cores
    return run_bass_kernel_spmd(_COMPILED["nc"], in_maps, list(range(N_CORES)),
                                **kw)


def kernel(**inputs):
    x = np.ascontiguousarray(np.asarray(inputs["x"], np.float32))
    edge_index = np.asarray(inputs["edge_index"])
    src = edge_index[0].astype(np.int64)
    dst = edge_index[1].astype(np.int64)

    prep = _prep_edges(src, dst)
    if prep is None:
        return _host_fallback(x, src, dst, inputs)
    eidx = prep["eidx"]
    ws = _prep_weights(inputs)

    in_maps = []
    for c in range(N_CORES):
        xt = np.zeros((131, LPAD), BF16)
        xt[:130, :LOC] = x[c * LOC:(c + 1) * LOC].T.astype(BF16)
        xt[130, :] = 1.0
        im = {"xt": xt, "eidx": eidx[c]}
        im.update(ws)
        in_maps.append(im)
    globals()["_LAST_IN_MAPS"] = in_maps
    globals()["_LAST_KEY"] = (prep["caw"], prep["cbw"])

    import time as _time
    t0 = _time.time()
    res = _run_device(in_maps, prep["caw"], prep["cbw"])
    globals()["_DEVICE_WALL_NS"] = int((_time.time() - t0) * 1e9)
    globals()["_LAST_RESULTS"] = res

    out = np.empty((N_NODES, 40), np.float32)
    for c in range(N_CORES):
        out[c * LOC:(c + 1) * LOC] = res.results[c]["out"][:LOC]
    return out


# revision 21
# speedup vs baseline: 1.0042x; 1.0042x over previous
"""Trainium2 kernel for nn_Net_68994354643186 (3-layer TransformerConv GNN).

Strategy (8 NeuronCores, node/data-parallel, edge-cut partitioning):
  - Nodes are partitioned into 8 shards of 6250 (padded to 6272 = 49*128).
  - Edges are owned by the core that owns their *destination* node, grouped
    into 49 windows of 128 destination nodes per core, with fixed padded
    capacities (13 tiles of "lo" src + 7 tiles of "hi" src per window; lo/hi
    split because dma_gather indices are int16).
  - ONE SPMD NEFF runs the whole network:
      per layer: replicated GEMM computes the k|v table for ALL nodes
      (avoids halo exchange for source gathers), a local GEMM computes q|s
      for the core's own nodes; the edge phase gathers k|v rows by src and
      q rows by dst (batched SWDGE dma_gather), computes exp(q.k/sqrt(D))
      per edge (no max subtraction -- scores are O(6) for this data), and
      aggregates numerator and denominator per destination node with
      selection-matrix matmuls accumulated in PSUM; normalization, root
      skip and leaky-relu happen at the node level.  Between layers the
      (transposed) node features are AllGathered so the next layer's
      replicated GEMM sees all nodes.  log_softmax of layer 3 runs on
      device; host only concatenates the 8 output shards.

Self-contained: hardcodes all shapes; no sibling imports.
"""

import sys

sys.path.insert(0, "/opt/trn_rl_repo")

import numpy as np
import ml_dtypes

BF16 = ml_dtypes.bfloat16

N_NODES = 50000
N_EDGES = 800000
N_CORES = 8
LOC = N_NODES // N_CORES          # 6250 local nodes per core
NW = 49                           # windows of 128 dst nodes per core
LPAD = NW * 128                   # 6272 padded local nodes
SPLIT = 32768                     # int16 index split for the kv table
CA_T, CB_T = 13, 7                # window capacity: lo/hi tiles of 128 edges
CAP_A, CAP_B = CA_T * 128, CB_T * 128
NT = CA_T + CB_T                  # 20 edge tiles per window
CAP = NT * 128                    # 2560 edge slots per window
LEAKY_ALPHA = 0.1

# (cin, H, D, hd, kv_pad_row_f32, q_pad_row_f32)
# kv_pad/q_pad are row lengths in bf16 elements; row bytes must be a
# multiple of 256 for dma_gather.
LAYERS = [
    (130, 4, 50, 200, 512, 256),
    (200, 4, 25, 100, 256, 128),
    (100, 4, 10, 40, 128, 128),
]

_COMPILED = {}


# --------------------------------------------------------------------------
# device program
# --------------------------------------------------------------------------

def _build_program(caw, cbw):
    import concourse.bass as bass
    import concourse.bacc as bacc
    import concourse.mybir as mybir
    import concourse.tile as tile
    from concourse.masks import make_identity

    f32 = mybir.dt.float32
    bf16 = mybir.dt.bfloat16
    i16 = mybir.dt.int16
    i32 = mybir.dt.int32

    nc = bacc.Bacc("TRN2", num_devices=N_CORES, num_swdge_queues=4)

    xT = nc.dram_tensor("xt", [131, LPAD], bf16, kind="ExternalInput")
    w_in = {}
    for li, (cin, H, D, hd, kvp, qp) in enumerate(LAYERS):
        w_in[("kv", li)] = nc.dram_tensor(
            f"wkv{li}", [cin + 1, 2 * hd], bf16, kind="ExternalInput")
        w_in[("qs", li)] = nc.dram_tensor(
            f"wqs{li}", [cin + 1, 2 * hd], bf16, kind="ExternalInput")
    # per-window column offsets into the packed edge-index tensor:
    # [kvlo (caw*8) | kvhi (cbw*8) | q (ntw*8) | widx (ntw, bf16 bits)]
    ntw = [a + b for a, b in zip(caw, cbw)]
    wcols = [a * 8 + b * 8 + n * 8 + n for a, b, n in zip(caw, cbw, ntw)]
    woff = [0]
    for v in wcols:
        woff.append(woff[-1] + v)
    NIDX = woff[-1]
    NTMAX = max(ntw)
    eidx_in = nc.dram_tensor("eidx", [128, NIDX], i16, kind="ExternalInput")
    out_t = nc.dram_tensor("out", [LPAD, 40], f32, kind="ExternalOutput")

    RG = [list(range(N_CORES))]

    with tile.TileContext(nc) as tc:
        with (
            tc.tile_pool(name="const", bufs=1) as cpool,
            tc.tile_pool(name="wpool", bufs=1) as wpool,
            tc.tile_pool(name="lhs", bufs=4) as lhs,
            tc.tile_pool(name="gout", bufs=4) as gout,
            tc.tile_pool(name="kvw", bufs=3) as kvw,
            tc.tile_pool(name="qw", bufs=3) as qw,
            tc.tile_pool(name="sw", bufs=3) as sw,
            tc.tile_pool(name="cw", bufs=3) as cw,
            tc.tile_pool(name="how", bufs=3) as how,
            tc.tile_pool(name="psg", bufs=2, space="PSUM") as psg,
            tc.tile_pool(name="psa", bufs=4, space="PSUM") as psa,
            tc.tile_pool(name="pst", bufs=2, space="PSUM") as pst,
            tc.tile_pool(name="dram", bufs=1, space="DRAM") as dram,
        ):
            # constants
            iota_i = cpool.tile([128, 128], i32, tag="iota_i")
            nc.gpsimd.iota(iota_i[:], pattern=[[1, 128]], base=0, channel_multiplier=0)
            iota_f = cpool.tile([128, 128], bf16, tag="iota_f")
            nc.vector.tensor_copy(out=iota_f[:], in_=iota_i[:])
            ident = cpool.tile([128, 128], f32, tag="ident")
            make_identity(nc, ident[:])

            # persistent DRAM buffers
            kv_t = [dram.tile([N_CORES * LPAD, LAYERS[l][4]], bf16, tag=f"kv{l}",
                              name=f"kv{l}") for l in range(3)]
            q_t = [dram.tile([LPAD, LAYERS[l][5]], bf16, tag=f"q{l}", name=f"q{l}")
                   for l in range(3)]
            s_t = [dram.tile([LPAD, LAYERS[l][3]], f32, tag=f"s{l}", name=f"s{l}")
                   for l in range(3)]
            # transposed local features (+1 ones row) per layer boundary
            hT = [None,
                  dram.tile([LAYERS[1][0] + 1, LPAD], bf16, tag="hT1", name="hT1"),
                  dram.tile([LAYERS[2][0] + 1, LPAD], bf16, tag="hT2", name="hT2")]
            xg = [dram.tile([N_CORES, LAYERS[0][0] + 1, LPAD], bf16, tag="xg0",
                            name="xg0", addr_space="Shared"),
                  dram.tile([N_CORES, LAYERS[1][0] + 1, LPAD], bf16, tag="xg1",
                            name="xg1", addr_space="Shared"),
                  dram.tile([N_CORES, LAYERS[2][0] + 1, LPAD], bf16, tag="xg2",
                            name="xg2", addr_space="Shared")]
            xloc = dram.tile([131, LPAD], bf16, tag="xloc")

            # stage layer-1 features and AllGather them
            nc.sync.dma_start(out=xloc[:], in_=xT.ap())
            tc.strict_bb_all_engine_barrier()
            nc.gpsimd.collective_compute(
                "AllGather", mybir.AluOpType.bypass, replica_groups=RG,
                ins=[xloc[:].opt()], outs=[xg[0][:].opt()])
            tc.strict_bb_all_engine_barrier()

            for li, (cin, H, D, hd, kvp, qp) in enumerate(LAYERS):
                KF0 = min(cin + 1, 128)
                K1 = cin + 1 - KF0
                loc_buf = xT.ap() if li == 0 else hT[li][:]
                gath = xg[li]

                # ---- weights to SBUF ----
                wkv0 = wpool.tile([KF0, 2 * hd], bf16, tag="wkv0")
                nc.sync.dma_start(out=wkv0[:], in_=w_in[("kv", li)].ap()[0:KF0, :])
                wqs0 = wpool.tile([KF0, 2 * hd], bf16, tag="wqs0")
                nc.sync.dma_start(out=wqs0[:], in_=w_in[("qs", li)].ap()[0:KF0, :])
                if K1:
                    wkv1 = wpool.tile([K1, 2 * hd], bf16, tag="wkv1")
                    nc.sync.dma_start(out=wkv1[:], in_=w_in[("kv", li)].ap()[KF0:cin + 1, :])
                    wqs1 = wpool.tile([K1, 2 * hd], bf16, tag="wqs1")
                    nc.sync.dma_start(out=wqs1[:], in_=w_in[("qs", li)].ap()[KF0:cin + 1, :])

                # ---- k|v GEMM over ALL nodes (replicated); node tiles are
                # processed in pairs so the bf16 lhsT loads use 512B rows ----
                for c in range(N_CORES):
                    for tp in range(0, NW, 2):
                        npair = min(2, NW - tp)
                        cl = slice(tp * 128, (tp + npair) * 128)
                        a0 = lhs.tile([128, 256], bf16, tag="a0")
                        nc.sync.dma_start(out=a0[:KF0, :npair * 128],
                                          in_=gath[c, 0:KF0, cl])
                        if K1:
                            a1 = lhs.tile([K1, 256], bf16, tag="a1")
                            nc.sync.dma_start(out=a1[:, :npair * 128],
                                              in_=gath[c, KF0:cin + 1, cl])
                        kvs = 2 * hd if 2 * hd * 2 >= 512 else kvp
                        o = gout.tile([128, 2, kvs], bf16, tag="go")
                        for j in range(npair):
                            jc = slice(j * 128, (j + 1) * 128)
                            ps = psg.tile([128, 2 * hd], f32, tag="psg")
                            nc.tensor.matmul(ps[:], lhsT=a0[:KF0, jc], rhs=wkv0[:],
                                             start=True, stop=(K1 == 0))
                            if K1:
                                nc.tensor.matmul(ps[:], lhsT=a1[:, jc], rhs=wkv1[:],
                                                 start=False, stop=True)
                            nc.vector.tensor_copy(out=o[:, j, 0:2 * hd], in_=ps[:])
                        r0 = (c * NW + tp) * 128
                        nc.sync.dma_start(
                            out=kv_t[li][r0:r0 + npair * 128, 0:kvs].rearrange(
                                "(j p) n -> p j n", p=128),
                            in_=o[:, 0:npair, :])

                # ---- q|s GEMM over local nodes ----
                for tp in range(0, NW, 2):
                    npair = min(2, NW - tp)
                    cl = slice(tp * 128, (tp + npair) * 128)
                    b0 = lhs.tile([128, 256], bf16, tag="a0")
                    nc.sync.dma_start(out=b0[:KF0, :npair * 128],
                                      in_=loc_buf[0:KF0, cl])
                    if K1:
                        b1 = lhs.tile([K1, 256], bf16, tag="a1")
                        nc.sync.dma_start(out=b1[:, :npair * 128],
                                          in_=loc_buf[KF0:cin + 1, cl])
                    oq = gout.tile([128, 2, qp], bf16, tag="gq")
                    os_ = gout.tile([128, 2, hd], f32, tag="gs")
                    for j in range(npair):
                        jc = slice(j * 128, (j + 1) * 128)
                        ps = psg.tile([128, 2 * hd], f32, tag="psg")
                        nc.tensor.matmul(ps[:], lhsT=b0[:KF0, jc], rhs=wqs0[:],
                                         start=True, stop=(K1 == 0))
                        if K1:
                            nc.tensor.matmul(ps[:], lhsT=b1[:, jc], rhs=wqs1[:],
                                             start=False, stop=True)
                        nc.vector.tensor_copy(out=oq[:, j, 0:hd], in_=ps[:, 0:hd])
                        nc.vector.tensor_copy(out=os_[:, j, :], in_=ps[:, hd:2 * hd])
                    r0 = tp * 128
                    nc.sync.dma_start(
                        out=q_t[li][r0:r0 + npair * 128, :].rearrange(
                            "(j p) n -> p j n", p=128),
                        in_=oq[:, 0:npair, :])
                    nc.sync.dma_start(
                        out=s_t[li][r0:r0 + npair * 128, :].rearrange(
                            "(j p) n -> p j n", p=128),
                        in_=os_[:, 0:npair, :])

                # ---- edge phase: one window of 128 dst nodes at a time ----
                qn = [0]

                def nextq():
                    qn[0] += 1
                    return qn[0] % 4

                for w in range(NW):
                    CA_W, CB_W = caw[w], cbw[w]
                    NT_W = CA_W + CB_W
                    eix = sw.tile([128, (NTMAX * 17 + 1)], i16, tag="eix")
                    nc.sync.dma_start(out=eix[:, 0:wcols[w]],
                                      in_=eidx_in.ap()[:, woff[w]:woff[w + 1]])
                    c0, c1, c2 = CA_W * 8, (CA_W + CB_W) * 8, \
                        (CA_W + CB_W + NT_W) * 8
                    ilo = eix[:, 0:c0]
                    ihi = eix[:, c0:c1]
                    iq = eix[:, c1:c2]
                    wx = eix[:, c2:c2 + NT_W].bitcast(bf16)

                    # dma_gather is limited to <=1024 indices per call
                    # (SWDGE ring depth); issue in chunks of <=7 tiles.
                    GMAX = 7
                    kvt = kvw.tile([128, NTMAX, kvp], bf16, tag="kvt")
                    for t0 in range(0, CA_W, GMAX):
                        tn = min(GMAX, CA_W - t0)
                        nc.gpsimd.dma_gather(
                            kvt[:, t0:t0 + tn, :], kv_t[li][:],
                            ilo[:, t0 * 8:(t0 + tn) * 8], tn * 128, tn * 128, kvp,
                            queue_num=nextq())
                    for t0 in range(0, CB_W, GMAX):
                        tn = min(GMAX, CB_W - t0)
                        nc.gpsimd.dma_gather(
                            kvt[:, CA_W + t0:CA_W + t0 + tn, :],
                            kv_t[li][SPLIT:, :],
                            ihi[:, t0 * 8:(t0 + tn) * 8], tn * 128, tn * 128, kvp,
                            queue_num=nextq())
                    qt = qw.tile([128, NTMAX, qp], bf16, tag="qt")
                    for t0 in range(0, NT_W, GMAX):
                        tn = min(GMAX, NT_W - t0)
                        nc.gpsimd.dma_gather(
                            qt[:, t0:t0 + tn, :], q_t[li][:],
                            iq[:, t0 * 8:(t0 + tn) * 8], tn * 128, tn * 128, qp,
                            queue_num=nextq())

                    st = sw.tile([128, hd], f32, tag="st")
                    nc.sync.dma_start(out=st[:],
                                      in_=s_t[li][w * 128:(w + 1) * 128, :])

                    # scores = sum_d q*k per head; q is pre-scaled by 1/sqrt(D)
                    tmp = cw.tile([128, NTMAX, hd], bf16, tag="tmp")
                    nc.vector.tensor_mul(tmp[:, 0:NT_W, :], qt[:, 0:NT_W, 0:hd],
                                         kvt[:, 0:NT_W, 0:hd])
                    sc = cw.tile([128, NTMAX, H], f32, tag="sc")
                    nc.vector.reduce_sum(
                        out=sc[:, 0:NT_W, :, None],
                        in_=tmp[:, 0:NT_W, :].rearrange(
                            "p t (h d) -> p t h d", h=H),
                        axis=mybir.AxisListType.X)
                    # wvex = [ exp(s)*v | exp(s) ]
                    wvex = cw.tile([128, NTMAX, hd + 4], bf16, tag="wvex")
                    ex = wvex[:, 0:NT_W, hd:hd + 4]
                    nc.scalar.activation(ex, sc[:, 0:NT_W, :],
                                         mybir.ActivationFunctionType.Exp)
                    nc.vector.tensor_mul(
                        wvex[:, 0:NT_W, 0:hd].rearrange(
                            "p t (h d) -> p t h d", h=H),
                        kvt[:, 0:NT_W, hd:2 * hd].rearrange(
                            "p t (h d) -> p t h d", h=H),
                        ex[:, :, :, None].to_broadcast([128, NT_W, H, D]))
                    # selection matrix: sel[p, t, j] = (widx[p, t] == j)
                    sel = cw.tile([128, NTMAX, 128], bf16, tag="sel")
                    nc.vector.tensor_tensor(
                        out=sel[:, 0:NT_W, :],
                        in0=wx[:, :, None].to_broadcast([128, NT_W, 128]),
                        in1=iota_f[:, None, :].to_broadcast([128, NT_W, 128]),
                        op=mybir.AluOpType.is_equal)
                    # aggregate numerator and denominator per dst node
                    ps = psa.tile([128, hd + 4], f32, tag="psa")
                    for t in range(NT_W):
                        nc.tensor.matmul(ps[:], lhsT=sel[:, t, :],
                                         rhs=wvex[:, t, :],
                                         start=(t == 0), stop=(t == NT_W - 1))
                    den = how.tile([128, 4], f32, tag="den")
                    nc.vector.tensor_scalar(
                        out=den[:], in0=ps[:, hd:hd + 4], scalar1=1e-30,
                        scalar2=None, op0=mybir.AluOpType.add)
                    rec = how.tile([128, 4], f32, tag="rec")
                    nc.vector.reciprocal(out=rec[:], in_=den[:])
                    ho = how.tile([128, hd], f32, tag="ho")
                    nc.vector.tensor_mul(
                        ho[:].rearrange("p (h d) -> p h d", h=H),
                        ps[:, 0:hd].rearrange("p (h d) -> p h d", h=H),
                        rec[:, :, None].to_broadcast([128, H, D]))
                    nc.vector.tensor_add(ho[:], ho[:], st[:])

                    if li < 2:
                        # leaky relu, then transpose into hT for the next layer
                        lk = how.tile([128, hd], f32, tag="lk")
                        nc.vector.tensor_scalar_mul(lk[:], ho[:], LEAKY_ALPHA)
                        nc.vector.tensor_max(ho[:], ho[:], lk[:])
                        for c0 in range(0, hd, 128):
                            cn = min(128, hd - c0)
                            pt = pst.tile([128, 128], f32, tag="pt")
                            nc.tensor.transpose(pt[:cn, :], ho[:, c0:c0 + cn],
                                                ident[:])
                            tt = how.tile([128, 128], bf16, tag="tt")
                            nc.vector.tensor_copy(out=tt[:cn, :], in_=pt[:cn, :])
                            nc.sync.dma_start(
                                out=hT[li + 1][c0:c0 + cn, w * 128:(w + 1) * 128],
                                in_=tt[:cn, :])
                    else:
                        # log_softmax over the 40 output columns
                        mx = how.tile([128, 1], f32, tag="mx")
                        nc.vector.reduce_max(out=mx[:], in_=ho[:],
                                             axis=mybir.AxisListType.X)
                        z = how.tile([128, 40], f32, tag="z")
                        nc.vector.tensor_scalar(
                            out=z[:], in0=ho[:], scalar1=mx[:, 0:1], scalar2=None,
                            op0=mybir.AluOpType.subtract)
                        e = how.tile([128, 40], f32, tag="e")
                        ssum = how.tile([128, 1], f32, tag="ssum")
                        nc.scalar.activation(e[:], z[:],
                                             mybir.ActivationFunctionType.Exp,
                                             accum_out=ssum[:])
                        lg = how.tile([128, 1], f32, tag="lg")
                        nc.scalar.activation(lg[:], ssum[:],
                                             mybir.ActivationFunctionType.Ln)
                        zo = how.tile([128, 40], f32, tag="zo")
                        nc.vector.tensor_scalar(
                            out=zo[:], in0=z[:], scalar1=lg[:, 0:1], scalar2=None,
                            op0=mybir.AluOpType.subtract)
                        nc.sync.dma_start(
                            out=out_t.ap()[w * 128:(w + 1) * 128, :], in_=zo[:])

                if li < 2:
                    # ones row for the bias trick, then AllGather features
                    ones = sw.tile([128, NW], bf16, tag="ones")
                    nc.vector.memset(ones[:], 1.0)
                    nc.sync.dma_start(
                        out=hT[li + 1][hd, :].rearrange("(p c) -> p c", p=128),
                        in_=ones[:])
                    tc.strict_bb_all_engine_barrier()
                    nc.gpsimd.collective_compute(
                        "AllGather", mybir.AluOpType.bypass, replica_groups=RG,
                        ins=[hT[li + 1][:].opt()], outs=[xg[li + 1][:].opt()])
                    tc.strict_bb_all_engine_barrier()

    nc.compile()
    return nc


# --------------------------------------------------------------------------
# host-side preparation
# --------------------------------------------------------------------------

def _prep_edges(src, dst):
    """Build per-core gather indices / window metadata with exact per-window
    tile counts (max over cores, since the SPMD program is shared).  Returns
    None if the graph needs a host fallback."""
    core = dst // LOC
    ldst = dst - core * LOC
    w = ldst >> 7
    wid = (ldst & 127).astype(np.float32)
    srcp = (src // LOC) * LPAD + (src % LOC)
    hi = srcp >= SPLIT
    g = core * NW + w

    deg = np.bincount(dst, minlength=N_NODES)
    if (deg == 0).any():
        return None
    nlo = np.bincount(g[~hi], minlength=N_CORES * NW).reshape(N_CORES, NW)
    nhi = np.bincount(g[hi], minlength=N_CORES * NW).reshape(N_CORES, NW)
    caw = (nlo.max(axis=0) + 127) // 128      # lo tiles per window
    cbw = (nhi.max(axis=0) + 127) // 128      # hi tiles per window
    ntw = caw + cbw
    if ntw.max() > 24:
        return None

    order = np.lexsort((hi, g))
    gs = g[order]
    his = hi[order]
    srcs = srcp[order]
    wids = wid[order]
    lds = ldst[order]

    key = gs * 2 + his
    run_start = np.concatenate(([0], np.flatnonzero(np.diff(key)) + 1))
    starts_full = np.zeros(N_CORES * NW * 2, np.int64)
    starts_full[key[run_start]] = run_start
    rank = np.arange(len(key)) - starts_full[key]

    # per-window slot base offsets (edge slots, in the concatenated layout)
    capA = caw * 128
    capB = cbw * 128
    wbase = np.zeros(NW, np.int64)
    wbase[1:] = np.cumsum(capA + capB)[:-1]
    TOT = int((capA + capB).sum())          # total edge slots per core

    g_w = gs % NW
    slot = wbase[g_w] + np.where(his, capA[g_w] + rank, rank)
    c_arr = gs // NW

    IDX = np.zeros((N_CORES, TOT), np.int16)      # gather row index
    QID = np.zeros((N_CORES, TOT), np.int16)
    WIDX = np.full((N_CORES, TOT), -1.0, np.float32)
    IDX[c_arr, slot] = np.where(his, srcs - SPLIT, srcs).astype(np.int16)
    QID[c_arr, slot] = lds.astype(np.int16)
    WIDX[c_arr, slot] = wids

    def wrap16(a):
        # edge slots i -> [i % 16, i // 16], replicated over 8 part. groups
        n = a.shape[-1]
        a = a.reshape(N_CORES, n // 16, 16).transpose(0, 2, 1)
        a = np.broadcast_to(a[:, None, :, :], (N_CORES, 8, 16, n // 16))
        return a.reshape(N_CORES, 128, n // 16)

    # pack per-window [kvlo | kvhi | q | widx] column blocks
    cols = []
    for w_ in range(NW):
        s0, e0 = wbase[w_], wbase[w_] + capA[w_]
        e1 = e0 + capB[w_]
        cols.append(wrap16(IDX[:, s0:e0]))
        cols.append(wrap16(IDX[:, e0:e1]))
        cols.append(wrap16(QID[:, s0:e1]))
        wx = WIDX[:, s0:e1].reshape(N_CORES, ntw[w_], 128).transpose(0, 2, 1)
        cols.append(np.ascontiguousarray(wx).astype(BF16).view(np.int16))
    eidx = np.ascontiguousarray(np.concatenate(cols, axis=2))
    return {"eidx": eidx, "caw": tuple(int(v) for v in caw),
            "cbw": tuple(int(v) for v in cbw)}


def _prep_weights(inputs):
    ws = {}
    for li, (cin, H, D, hd, kvp, qp) in enumerate(LAYERS):
        s = 1.0 / np.sqrt(np.float32(D))
        wkv = np.zeros((cin + 1, 2 * hd), np.float32)
        wkv[:cin, :hd] = inputs[f"Wk{li + 1}"]
        wkv[cin, :hd] = inputs[f"bk{li + 1}"]
        wkv[:cin, hd:] = inputs[f"Wv{li + 1}"]
        wkv[cin, hd:] = inputs[f"bv{li + 1}"]
        wqs = np.zeros((cin + 1, 2 * hd), np.float32)
        wqs[:cin, :hd] = inputs[f"Wq{li + 1}"] * s
        wqs[cin, :hd] = inputs[f"bq{li + 1}"] * s
        wqs[:cin, hd:] = inputs[f"Ws{li + 1}"]
        wqs[cin, hd:] = inputs[f"bs{li + 1}"]
        ws[f"wkv{li}"] = wkv.astype(BF16)
        ws[f"wqs{li}"] = wqs.astype(BF16)
    return ws


def _host_fallback(x, src, dst, inputs):
    """Pure-numpy reference path (used only if the graph exceeds the
    compiled capacities)."""
    order = np.argsort(dst, kind="stable")
    so, do = src[order], dst[order]
    seg_starts = np.flatnonzero(np.concatenate(([True], do[1:] != do[:-1])))
    seg_ids = do[seg_starts]
    h = x
    for li, (cin, H, D, hd, kvp, qp) in enumerate(LAYERS):
        q = (h @ inputs[f"Wq{li + 1}"] + inputs[f"bq{li + 1}"]).reshape(-1, H, D)
        k = (h @ inputs[f"Wk{li + 1}"] + inputs[f"bk{li + 1}"]).reshape(-1, H, D)
        v = (h @ inputs[f"Wv{li + 1}"] + inputs[f"bv{li + 1}"]).reshape(-1, H, D)
        s = h @ inputs[f"Ws{li + 1}"] + inputs[f"bs{li + 1}"]
        sc = np.einsum("ehd,ehd->eh", q[do], k[so], optimize=True) / np.sqrt(
            np.float32(D))
        m = np.zeros((N_NODES, H), np.float32)
        m[seg_ids] = np.maximum.reduceat(sc, seg_starts, axis=0)
        e = np.exp(sc - m[do])
        den = np.zeros((N_NODES, H), np.float32)
        den[seg_ids] = np.add.reduceat(e, seg_starts, axis=0)
        alpha = e / (den[do] + 1e-16)
        outa = np.zeros((N_NODES, H, D), np.float32)
        outa[seg_ids] = np.add.reduceat(alpha[:, :, None] * v[so], seg_starts,
                                        axis=0)
        h = outa.reshape(N_NODES, hd) + s
        if li < 2:
            h = np.where(h >= 0, h, np.float32(LEAKY_ALPHA) * h)
    m = h.max(axis=1, keepdims=True)
    z = h - m
    return (z - np.log(np.exp(z).sum(axis=1, keepdims=True))).astype(np.float32)


def _run_device(in_maps, caw, cbw, trace=False, trace_cores=None):
    from concourse.bass_utils import run_bass_kernel_spmd

    key = (tuple(caw), tuple(cbw))
    if _COMPILED.get("key") != key:
        _COMPILED["nc"] = _build_program(caw, cbw)
        _COMPILED["key"] = key
    kw = {}
    if trace:
        kw = dict(trace=True)
        if trace_cores is not None:
            kw["trace_cores"] = trace_cores
    return run_bass_kernel_spmd(_COMPILED["nc"], in_maps, list(range(N_CORES)),
                                **kw)


def kernel(**inputs):
    x = np.ascontiguousarray(np.asarray(inputs["x"], np.float32))
    edge_index = np.asarray(inputs["edge_index"])
    src = edge_index[0].astype(np.int64)
    dst = edge_index[1].astype(np.int64)

    prep = _prep_edges(src, dst)
    if prep is None:
        return _host_fallback(x, src, dst, inputs)
    eidx = prep["eidx"]
    ws = _prep_weights(inputs)

    in_maps = []
    for c in range(N_CORES):
        xt = np.zeros((131, LPAD), BF16)
        xt[:130, :LOC] = x[c * LOC:(c + 1) * LOC].T.astype(BF16)
        xt[130, :] = 1.0
        im = {"xt": xt, "eidx": eidx[c]}
        im.update(ws)
        in_maps.append(im)
    globals()["_LAST_IN_MAPS"] = in_maps
    globals()["_LAST_KEY"] = (prep["caw"], prep["cbw"])

    import time as _time
    t0 = _time.time()
    res = _run_device(in_maps, prep["caw"], prep["cbw"])
    globals()["_DEVICE_WALL_NS"] = int((_time.time() - t0) * 1e9)
    globals()["_LAST_RESULTS"] = res

    out = np.empty((N_NODES, 40), np.float32)
    for c in range(N_CORES):
        out[c * LOC:(c + 1) * LOC] = res.results[c]["out"][:LOC]
    return out
